# revision 5
# baseline (speedup 1.0000x reference)
"""DWT 2x2 low-low pooling (bior1.3) for Trainium2, 8-core data parallel.

The reference banded matrices reduce to: out[b,c,l,k] =
0.5 * (x[2l,2k] + x[2l,2k+1] + x[2l+1,2k] + x[2l+1,2k+1])
i.e. a scaled 2x2 sum pool.  Memory-bound: per core we stream 32 MiB in,
8 MiB out.

Layout per core: B*C = 1024 images of [256,256]; each core takes 128
contiguous images.  A supertile groups G images as an SBUF tile
[128 part = h/2, G, 2 (row parity), 256] -- each partition holds the two
rows of one output row, contiguous 2 KiB DMA runs.  DVE pass 1 adds row
pairs; DVE pass 2 (tensor_tensor_reduce) adds column pairs fused with the
0.5 scale.  Output DMA'd from the ACT HWDGE ring so it doesn't queue
behind input DMAs on the SP ring.
"""

import sys

sys.path.insert(0, "/opt/trn_rl_repo")

import numpy as np

import concourse.bacc as bacc
import concourse.bass as bass
import concourse.tile as tile
from concourse import mybir
from concourse.bass_utils import run_bass_kernel_spmd

N_CORES = 8
B, C, H, W = 16, 64, 256, 256
IMGS = B * C  # 1024
IMGS_PER_CORE = IMGS // N_CORES  # 128
G = 16  # images per supertile
F32 = mybir.dt.float32


def build(n_img=IMGS_PER_CORE, g=G, in_bufs=2):
    nc = bacc.Bacc(
        "TRN2", target_bir_lowering=False, debug=False, num_devices=N_CORES
    )
    x = nc.dram_tensor("x", [n_img, H, W], F32, kind="ExternalInput").ap()
    out = nc.dram_tensor(
        "out", [n_img, H // 2, W // 2], F32, kind="ExternalOutput"
    ).ap()
    ng = n_img // g
    hp = H // 2  # 128 partitions
    with tile.TileContext(nc) as tc:
        with (
            tc.tile_pool(name="pin", bufs=in_bufs) as pin,
            tc.tile_pool(name="ps", bufs=2) as ps,
            tc.tile_pool(name="po", bufs=2) as po,
        ):
            for gi in range(ng):
                xg = x[gi * g : (gi + 1) * g].rearrange(
                    "i (hp p2) w -> hp i p2 w", p2=2
                )
                tin = pin.tile([hp, g, 2, W], F32)
                nc.sync.dma_start(out=tin[:, :, :, :], in_=xg)

                s = ps.tile([hp, g, W], F32)
                nc.vector.tensor_add(
                    s[:, :, :], tin[:, :, 0, :], tin[:, :, 1, :]
                )

                o = po.tile([hp, g, W // 2], F32)
                sv = s.rearrange("p i (k q) -> p i k q", q=2)
                nc.vector.tensor_add(
                    o[:, :, :], sv[:, :, :, 0], sv[:, :, :, 1]
                )
                nc.gpsimd.tensor_scalar_mul(o[:, :, :], o[:, :, :], 0.5)

                og = out[gi * g : (gi + 1) * g].rearrange("i hp k -> hp i k")
                nc.scalar.dma_start(out=og, in_=o[:, :, :])
    nc.compile()
    return nc


def _forward(x, trace=False):
    x = np.ascontiguousarray(x, dtype=np.float32).reshape(IMGS, H, W)
    nc = build()
    core_ids = list(range(N_CORES))
    in_maps = [
        {"x": np.ascontiguousarray(x[c * IMGS_PER_CORE : (c + 1) * IMGS_PER_CORE])}
        for c in core_ids
    ]
    r = run_bass_kernel_spmd(nc, in_maps, core_ids, trace=trace)
    out = np.concatenate([r.results[c]["out"] for c in core_ids], axis=0)
    return out.reshape(B, C, H // 2, W // 2), r


def kernel(x):
    out, _ = _forward(x, trace=False)
    return out


# revision 6
# speedup vs baseline: 2.1389x; 2.1389x over previous
"""DWT 2x2 low-low pooling (bior1.3) for Trainium2, 8-core data parallel.

The reference banded matrices reduce to: out[b,c,l,k] =
0.5 * (x[2l,2k] + x[2l,2k+1] + x[2l+1,2k] + x[2l+1,2k+1])
i.e. a scaled 2x2 sum pool.  Memory-bound: per core we stream 32 MiB in,
8 MiB out.

Layout per core: B*C = 1024 images of [256,256]; each core takes 128
contiguous images.  A supertile groups G images as an SBUF tile
[128 part = h/2, G, 2 (row parity), 256] -- each partition holds the two
rows of one output row, contiguous 2 KiB DMA runs.  DVE pass 1 adds row
pairs; DVE pass 2 (tensor_tensor_reduce) adds column pairs fused with the
0.5 scale.  Output DMA'd from the ACT HWDGE ring so it doesn't queue
behind input DMAs on the SP ring.
"""

import sys

sys.path.insert(0, "/opt/trn_rl_repo")

import numpy as np

import concourse.bacc as bacc
import concourse.bass as bass
import concourse.tile as tile
from concourse import mybir
from concourse.bass_utils import run_bass_kernel_spmd

N_CORES = 8
B, C, H, W = 16, 64, 256, 256
IMGS = B * C  # 1024
IMGS_PER_CORE = IMGS // N_CORES  # 128
G = 16  # images per supertile
F32 = mybir.dt.float32


def build(n_img=IMGS_PER_CORE, g=G, in_bufs=2):
    nc = bacc.Bacc(
        "TRN2", target_bir_lowering=False, debug=False, num_devices=N_CORES
    )
    x = nc.dram_tensor("x", [n_img, H, W], F32, kind="ExternalInput").ap()
    out = nc.dram_tensor(
        "out", [n_img, H // 2, W // 2], F32, kind="ExternalOutput"
    ).ap()
    ng = n_img // g
    hp = H // 2  # 128 partitions
    with tile.TileContext(nc) as tc:
        with (
            tc.tile_pool(name="pin", bufs=in_bufs) as pin,
            tc.tile_pool(name="ps", bufs=2) as ps,
            tc.tile_pool(name="po", bufs=2) as po,
        ):
            for gi in range(ng):
                xg = x[gi * g : (gi + 1) * g].rearrange(
                    "i (hp p2) w -> hp i p2 w", p2=2
                )
                tin = pin.tile([hp, g, 2, W], F32)
                nc.sync.dma_start(out=tin[:, :, :, :], in_=xg)

                s = ps.tile([hp, g, W], F32)
                nc.vector.tensor_add(
                    s[:, :, :], tin[:, :, 0, :], tin[:, :, 1, :]
                )

                o = po.tile([hp, g, W // 2], F32)
                sv = s.rearrange("p i (k q) -> p i k q", q=2)
                nc.vector.tensor_add(
                    o[:, :, :], sv[:, :, :, 0], sv[:, :, :, 1]
                )
                o2 = po.tile([hp, g, W // 2], F32, tag="o2")
                nc.scalar.mul(o2[:, :, :], o[:, :, :], 0.5)

                og = out[gi * g : (gi + 1) * g].rearrange("i hp k -> hp i k")
                nc.scalar.dma_start(out=og, in_=o2[:, :, :])
    nc.compile()
    return nc


def _forward(x, trace=False):
    x = np.ascontiguousarray(x, dtype=np.float32).reshape(IMGS, H, W)
    nc = build()
    core_ids = list(range(N_CORES))
    in_maps = [
        {"x": np.ascontiguousarray(x[c * IMGS_PER_CORE : (c + 1) * IMGS_PER_CORE])}
        for c in core_ids
    ]
    r = run_bass_kernel_spmd(nc, in_maps, core_ids, trace=trace)
    out = np.concatenate([r.results[c]["out"] for c in core_ids], axis=0)
    return out.reshape(B, C, H // 2, W // 2), r


def kernel(x):
    out, _ = _forward(x, trace=False)
    return out


# revision 7
# speedup vs baseline: 2.2554x; 1.0545x over previous
"""DWT 2x2 low-low pooling (bior1.3) for Trainium2, 8-core data parallel.

The reference banded matrices reduce to: out[b,c,l,k] =
0.5 * (x[2l,2k] + x[2l,2k+1] + x[2l+1,2k] + x[2l+1,2k+1])
i.e. a scaled 2x2 sum pool.  Memory-bound: per core we stream 32 MiB in,
8 MiB out.

Layout per core: B*C = 1024 images of [256,256]; each core takes 128
contiguous images.  A supertile groups G images as an SBUF tile
[128 part = h/2, G, 2 (row parity), 256] -- each partition holds the two
rows of one output row, contiguous 2 KiB DMA runs.  DVE pass 1 adds row
pairs; DVE pass 2 (tensor_tensor_reduce) adds column pairs fused with the
0.5 scale.  Output DMA'd from the ACT HWDGE ring so it doesn't queue
behind input DMAs on the SP ring.
"""

import sys

sys.path.insert(0, "/opt/trn_rl_repo")

import numpy as np

import concourse.bacc as bacc
import concourse.bass as bass
import concourse.tile as tile
from concourse import mybir
from concourse.bass_utils import run_bass_kernel_spmd

N_CORES = 8
B, C, H, W = 16, 64, 256, 256
IMGS = B * C  # 1024
IMGS_PER_CORE = IMGS // N_CORES  # 128
G = 16  # images per supertile
F32 = mybir.dt.float32


def _group_sizes(n_img, g):
    """Small groups at the head (fast pipeline fill) and tail (early
    final DMA), big groups in the middle (DMA efficiency)."""
    if n_img != IMGS_PER_CORE or g != G:
        return [g] * (n_img // g)
    sizes = [4, 4, 8] + [16] * 6 + [8, 4, 4]
    assert sum(sizes) == n_img
    return sizes


def build(n_img=IMGS_PER_CORE, g=G, in_bufs=3):
    nc = bacc.Bacc(
        "TRN2", target_bir_lowering=False, debug=False, num_devices=N_CORES
    )
    x = nc.dram_tensor("x", [n_img, H, W], F32, kind="ExternalInput").ap()
    out = nc.dram_tensor(
        "out", [n_img, H // 2, W // 2], F32, kind="ExternalOutput"
    ).ap()
    hp = H // 2  # 128 partitions
    with tile.TileContext(nc) as tc:
        with (
            tc.tile_pool(name="pin", bufs=in_bufs) as pin,
            tc.tile_pool(name="ps", bufs=2) as ps,
            tc.tile_pool(name="po", bufs=2) as po,
        ):
            i0 = 0
            for gs in _group_sizes(n_img, g):
                xg = x[i0 : i0 + gs].rearrange("i (hp p2) w -> hp i p2 w", p2=2)
                tin = pin.tile([hp, gs, 2, W], F32, tag="tin")
                nc.sync.dma_start(out=tin[:, :, :, :], in_=xg)

                s = ps.tile([hp, gs, W], F32, tag="s")
                nc.vector.tensor_add(
                    s[:, :, :], tin[:, :, 0, :], tin[:, :, 1, :]
                )

                o = po.tile([hp, gs, W // 2], F32, tag="o")
                sv = s.rearrange("p i (k q) -> p i k q", q=2)
                nc.vector.tensor_add(
                    o[:, :, :], sv[:, :, :, 0], sv[:, :, :, 1]
                )
                o2 = po.tile([hp, gs, W // 2], F32, tag="o2")
                nc.scalar.mul(o2[:, :, :], o[:, :, :], 0.5)

                og = out[i0 : i0 + gs].rearrange("i hp k -> hp i k")
                nc.scalar.dma_start(out=og, in_=o2[:, :, :])
                i0 += gs
    nc.compile()
    return nc


def _forward(x, trace=False):
    x = np.ascontiguousarray(x, dtype=np.float32).reshape(IMGS, H, W)
    nc = build()
    core_ids = list(range(N_CORES))
    in_maps = [
        {"x": np.ascontiguousarray(x[c * IMGS_PER_CORE : (c + 1) * IMGS_PER_CORE])}
        for c in core_ids
    ]
    r = run_bass_kernel_spmd(nc, in_maps, core_ids, trace=trace)
    out = np.concatenate([r.results[c]["out"] for c in core_ids], axis=0)
    return out.reshape(B, C, H // 2, W // 2), r


def kernel(x):
    out, _ = _forward(x, trace=False)
    return out


# revision 8
# speedup vs baseline: 2.4931x; 1.1054x over previous
"""DWT 2x2 low-low pooling (bior1.3) for Trainium2, 8-core data parallel.

The reference banded matrices reduce to: out[b,c,l,k] =
0.5 * (x[2l,2k] + x[2l,2k+1] + x[2l+1,2k] + x[2l+1,2k+1])
i.e. a scaled 2x2 sum pool.  Memory-bound: per core we stream 32 MiB in,
8 MiB out.

Layout per core: B*C = 1024 images of [256,256]; each core takes 128
contiguous images.  A supertile groups G images as an SBUF tile
[128 part = h/2, G, 2 (row parity), 256] -- each partition holds the two
rows of one output row, contiguous 2 KiB DMA runs.  DVE pass 1 adds row
pairs; DVE pass 2 (tensor_tensor_reduce) adds column pairs fused with the
0.5 scale.  Output DMA'd from the ACT HWDGE ring so it doesn't queue
behind input DMAs on the SP ring.
"""

import sys

sys.path.insert(0, "/opt/trn_rl_repo")

import numpy as np

import concourse.bacc as bacc
import concourse.bass as bass
import concourse.tile as tile
from concourse import mybir
from concourse.bass_utils import run_bass_kernel_spmd

N_CORES = 8
B, C, H, W = 16, 64, 256, 256
IMGS = B * C  # 1024
IMGS_PER_CORE = IMGS // N_CORES  # 128
G = 16  # images per supertile
F32 = mybir.dt.float32


def _group_sizes(n_img, g):
    """Small groups at the head (fast pipeline fill) and tail (early
    final DMA), big groups in the middle (DMA efficiency)."""
    if n_img != IMGS_PER_CORE or g != G:
        return [g] * (n_img // g)
    sizes = [4, 4] + [8] * 14 + [4, 4]
    assert sum(sizes) == n_img
    return sizes


def build(n_img=IMGS_PER_CORE, g=G, in_bufs=4):
    nc = bacc.Bacc(
        "TRN2", target_bir_lowering=False, debug=False, num_devices=N_CORES
    )
    x = nc.dram_tensor("x", [n_img, H, W], F32, kind="ExternalInput").ap()
    out = nc.dram_tensor(
        "out", [n_img, H // 2, W // 2], F32, kind="ExternalOutput"
    ).ap()
    hp = H // 2  # 128 partitions
    with tile.TileContext(nc) as tc:
        with (
            tc.tile_pool(name="pin", bufs=in_bufs) as pin,
            tc.tile_pool(name="ps", bufs=2) as ps,
            tc.tile_pool(name="po", bufs=2) as po,
        ):
            i0 = 0
            for gs in _group_sizes(n_img, g):
                xg = x[i0 : i0 + gs].rearrange("i (hp p2) w -> hp i p2 w", p2=2)
                tin = pin.tile([hp, gs, 2, W], F32, tag="tin")
                nc.sync.dma_start(out=tin[:, :, :, :], in_=xg)

                s = ps.tile([hp, gs, W], F32, tag="s")
                nc.vector.tensor_add(
                    s[:, :, :], tin[:, :, 0, :], tin[:, :, 1, :]
                )

                o = po.tile([hp, gs, W // 2], F32, tag="o")
                sv = s.rearrange("p i (k q) -> p i k q", q=2)
                nc.vector.tensor_add(
                    o[:, :, :], sv[:, :, :, 0], sv[:, :, :, 1]
                )
                o2 = po.tile([hp, gs, W // 2], F32, tag="o2")
                nc.scalar.mul(o2[:, :, :], o[:, :, :], 0.5)

                og = out[i0 : i0 + gs].rearrange("i hp k -> hp i k")
                nc.scalar.dma_start(out=og, in_=o2[:, :, :])
                i0 += gs
    nc.compile()
    return nc


def _forward(x, trace=False):
    x = np.ascontiguousarray(x, dtype=np.float32).reshape(IMGS, H, W)
    nc = build()
    core_ids = list(range(N_CORES))
    in_maps = [
        {"x": np.ascontiguousarray(x[c * IMGS_PER_CORE : (c + 1) * IMGS_PER_CORE])}
        for c in core_ids
    ]
    r = run_bass_kernel_spmd(nc, in_maps, core_ids, trace=trace)
    out = np.concatenate([r.results[c]["out"] for c in core_ids], axis=0)
    return out.reshape(B, C, H // 2, W // 2), r


def kernel(x):
    out, _ = _forward(x, trace=False)
    return out
